# revision 1
# baseline (speedup 1.0000x reference)
"""LSTM encoder (T=512, B=256, H=256, V=32000) on 8 trn2 NeuronCores.

Strategy
--------
Data-parallel over batch: B=256 -> 32 per core; weights/table replicated.

Per core the recurrence runs in a transposed "gatesT" layout: gates live as
[4H on partitions (8 chunks of 128), batch in the free dim]. Weight chunks
are the stationary matmul operand (fp16), h streams as the moving operand.

The per-core batch of 32 is split into TWO independent 16-lane chains that
run interleaved: while chain A is in its sigmoid/elementwise tail, chain B
owns the PE, and vice versa. This hides the large fixed per-instruction
latencies (ACT ~350ns, DVE ~190ns) that dominate a single serial
recurrence chain, which is otherwise latency-bound at ~3us/step.

The input projections W_ih @ emb_t for a window of 8 future steps are
precomputed into the window's PSUM banks by 128-column matmuls (spread
over the window's steps so they never head-of-line-block the recurrent
burst); the per-step W_hh matmuls accumulate straight on top
(start=False). The first write to each PSUM bank is a single full-bank
N=512 bias matmul with start=True (start zeroes the whole 2KB bank).

Embeddings are fetched with dma_gather(transpose=True): gathers fp16 table
rows and deposits them H-on-partitions, the exact rhs layout the
X-projection matmuls need.

Gate chunk order is permuted host-side to [f, i, o, g] and the g-gate rows
are pre-scaled by 2 so ONE sigmoid call covers all four gates
(tanh(x) = 2*sigmoid(2x) - 1); the tanh correction folds into one DVE
tensor_scalar op, and one fused DVE mul computes [f*c, i*tg].

Numerics: fp16 table/weights/h (matmul operands), fp32 PSUM and fp32
elementwise state c. Host emulation of this scheme measures absmax/scale
error ~4.5e-4 vs the fp32 reference.
"""

import numpy as np

T, B, H, V = 512, 256, 256, 32000
N_CORES = 8
BL = B // N_CORES          # 32 batch per core
HB = BL // 2               # 16 per half-chain
S = 8                      # steps per PSUM window
G4 = 4 * H                 # 1024
M = G4 // 128              # 8 gate chunks
K = H // 128               # 2 contraction chunks

# gate chunk order f, i, o, g (PyTorch native is i, f, g, o)
_PERM = np.concatenate([
    np.arange(H, 2 * H),       # f
    np.arange(0, H),           # i
    np.arange(3 * H, 4 * H),   # o
    np.arange(2 * H, 3 * H),   # g
])


def _build_bass(t_steps=T):
    from contextlib import ExitStack
    from concourse import bacc, mybir, library_config
    import concourse.tile as tile

    f16, f32, i16 = mybir.dt.float16, mybir.dt.float32, mybir.dt.int16
    Sig = mybir.ActivationFunctionType.Sigmoid
    Tanh = mybir.ActivationFunctionType.Tanh
    mult, add = mybir.AluOpType.mult, mybir.AluOpType.add
    sub = mybir.AluOpType.subtract

    NW = t_steps // S
    NI = S * BL            # 256 gathered rows per window (both halves)

    nc = bacc.Bacc("TRN2", target_bir_lowering=False, debug=False)
    idx_d = nc.declare_dram_parameter("idx", [128, NW, NI // 16], i16, isOutput=False)
    tab_d = nc.declare_dram_parameter("table", [V, H], f16, isOutput=False)
    wih_d = nc.declare_dram_parameter("wih_t", [H, G4], f16, isOutput=False)
    whh_d = nc.declare_dram_parameter("whh_t", [H, G4], f16, isOutput=False)
    b_d = nc.declare_dram_parameter("bias", [4, G4 // 4], f16, isOutput=False)
    ind_d = nc.declare_dram_parameter("ind", [4, 4 * S * HB], f16, isOutput=False)
    h0_d = nc.declare_dram_parameter("h0t", [128, K, BL], f16, isOutput=False)
    c0_d = nc.declare_dram_parameter("c0t", [128, K, BL], f32, isOutput=False)
    ho_d = nc.declare_dram_parameter("h_out", [128, K, BL], f32, isOutput=True)
    co_d = nc.declare_dram_parameter("c_out", [128, K, BL], f32, isOutput=True)

    import bass_rust

    with tile.TileContext(nc) as tc, ExitStack() as ctx:
        const = ctx.enter_context(tc.tile_pool(name="const", bufs=1))
        embp = ctx.enter_context(tc.tile_pool(name="embp", bufs=3))
        psum = ctx.enter_context(tc.tile_pool(name="psum", bufs=2, space="PSUM"))
        sp = ctx.enter_context(tc.tile_pool(name="sp", bufs=3))
        tmp = ctx.enter_context(tc.tile_pool(name="tmp", bufs=3))
        hp = ctx.enter_context(tc.tile_pool(name="hp", bufs=3))

        # idx upload + library load first so the first gather's Q7 work
        # overlaps the remaining constant DMAs
        idx_sb = const.tile([128, NW, NI // 16], i16, name="idx_sb")
        nc.sync.dma_start(idx_sb[:], idx_d[:])
        nc.gpsimd.load_library(library_config.mlp)
        whh_sb, wih_sb = [], []
        for k in range(K):
            wt = const.tile([128, G4], f16, name=f"whh_sb{k}")
            nc.sync.dma_start(wt[:], whh_d[128 * k:128 * (k + 1), :])
            whh_sb.append(wt)
            xt = const.tile([128, G4], f16, name=f"wih_sb{k}")
            nc.sync.dma_start(xt[:], wih_d[128 * k:128 * (k + 1), :])
            wih_sb.append(xt)
        b_sb = const.tile([4, G4 // 4], f16, name="b_sb")
        nc.sync.dma_start(b_sb[:], b_d[:])
        ind = const.tile([4, 4 * S * HB], f16, name="ind")
        nc.sync.dma_start(ind[:], ind_d[:])

        # per-half state: ct = [c (2 chunks) | tg (2 chunks)], h
        ct, h_cur = [], []
        for ha in range(2):
            c_t = const.tile([128, 2 * K, HB], f32, name=f"ct{ha}")
            nc.sync.dma_start(c_t[:, 0:K, :], c0_d[:, :, HB * ha:HB * (ha + 1)])
            ct.append(c_t)
            h0 = const.tile([128, K, HB], f16, name=f"h0_sb{ha}")
            nc.sync.dma_start(h0[:], h0_d[:, :, HB * ha:HB * (ha + 1)])
            h_cur.append(h0)

        embt = {}
        ps = {}

        def gather(w):
            # one gather per window; half A = cols 0:128, half B = 128:256
            e = embp.tile([128, K, NI], f16, name="embt", tag=f"embt{w % 3}",
                          bufs=1)
            g_i = nc.gpsimd.dma_gather(
                out_ap=e[:], in_ap=tab_d[:],
                idxs_ap=idx_sb[:, w, :],
                num_idxs=NI, num_idxs_reg=NI, elem_size=H, transpose=True)
            embt[w] = e
            return g_i

        def bias_mms(w, ha, after=None):
            # first write to each bank: full-bank N=512 matmul, start=True
            p = psum.tile([128, M, S, HB], f32, name="ps",
                          tag=f"ps{ha}_{w % 2}", bufs=1)
            for b in range(2):
                mm = nc.tensor.matmul(
                    out=p[:, 4 * b:4 * b + 4, :, :],
                    lhsT=b_sb[:, 128 * b:128 * (b + 1)],
                    rhs=ind[:], start=True, stop=False, skip_group_check=True)
                if after is not None:
                    bass_rust.add_dep_helper(mm.ins, after.ins, sync=False,
                                             reason="pin bias after burst")
            ps[(w, ha)] = p

        def x_mms(w, ha, lo, hi, after=None):
            for j in range(lo, hi):
                m, k = j // K, j % K
                mm = nc.tensor.matmul(
                    out=ps[(w, ha)][:, m, :, :],
                    lhsT=wih_sb[k][:, 128 * m:128 * (m + 1)],
                    rhs=embt[w][:, k, 128 * ha:128 * (ha + 1)],
                    start=False, stop=False, skip_group_check=True)
                if after is not None:
                    bass_rust.add_dep_helper(mm.ins, after.ins, sync=False,
                                             reason="pin x after burst")

        def burst(w, s, ha):
            last = None
            for k in range(K):
                for m in range(M):
                    last = nc.tensor.matmul(
                        out=ps[(w, ha)][:, m, s, :],
                        lhsT=whh_sb[k][:, 128 * m:128 * (m + 1)],
                        rhs=h_cur[ha][:, k, :],
                        start=False, stop=(k == K - 1), skip_group_check=True)
            return last

        # prologue: window 0 fully prepared, window 1 gathered
        gather(0)
        if NW > 1:
            gather(1)
        for ha in range(2):
            bias_mms(0, ha)
            x_mms(0, ha, 0, M * K)

        for w in range(NW):
            for s in range(S):
                t = w * S + s
                burst(w, s, 0)
                last_mm = burst(w, s, 1)
                sall, mm12h = [], []
                for ha in range(2):
                    sa = sp.tile([128, M, HB], f32, name="sall", tag=f"sall{ha}")
                    sall.append(sa)
                    mh = tmp.tile([128, 2 * K, HB], f32, name="mm12", tag=f"mm12{ha}")
                    mm12h.append(mh)
                # stage-interleaved emission so each engine FIFO alternates
                siga_i = nc.scalar.activation(
                    sall[0][:], ps[(w, 0)][:, :, s, :], Sig)
                nc.scalar.activation(sall[1][:], ps[(w, 1)][:, :, s, :], Sig)
                tch = [tmp.tile([128, K, HB], f32, name="tct", tag=f"tct{ha}")
                       for ha in range(2)]
                def c_update(ha):
                    return nc.vector.tensor_tensor(
                        ct[ha][:, 0:K, :], mm12h[ha][:, 0:K, :],
                        mm12h[ha][:, K:2 * K, :], add)

                def h_update(ha):
                    # returns the last emitted h instruction. Chain A gets a
                    # per-K-chunk split so its k=0 matmuls start one DVE op
                    # earlier; chain B has slack and uses one op.
                    if t < t_steps - 1:
                        hn = hp.tile([128, K, HB], f16, name="hn", tag=f"hn{ha}")
                        if ha == 0:
                            for k in range(K):
                                hk = nc.vector.tensor_tensor(
                                    hn[:, k, :], sall[ha][:, 4 + k, :],
                                    tch[ha][:, k, :], mult)
                        else:
                            hk = nc.vector.tensor_tensor(
                                hn[:], sall[ha][:, 4:6, :], tch[ha][:], mult)
                        h_cur[ha] = hn
                        return hk
                    hf = tmp.tile([128, K, HB], f32, name="hf", tag=f"hf{ha}")
                    hk = nc.vector.tensor_tensor(
                        hf[:], sall[ha][:, 4:6, :], tch[ha][:], mult)
                    nc.sync.dma_start(ho_d[:, :, HB * ha:HB * (ha + 1)], hf[:])
                    nc.sync.dma_start(co_d[:, :, HB * ha:HB * (ha + 1)],
                                      ct[ha][:, 0:K, :])
                    return hk

                c_i = None
                for ha in range(2):
                    # tg = 2*sig(2g) - 1 = tanh(g)
                    tg_i = nc.vector.tensor_scalar(
                        ct[ha][:, K:2 * K, :], sall[ha][:, 6:8, :], 2.0, 1.0,
                        mult, sub)
                    if c_i is not None:
                        # keep chain A's DVE trio contiguous in the FIFO
                        bass_rust.add_dep_helper(
                            tg_i.ins, c_i.ins, sync=False,
                            reason="chain B after chain A trio")
                    # fused [f*c, i*tg]
                    nc.vector.tensor_tensor(
                        mm12h[ha][:], sall[ha][:, 0:4, :], ct[ha][:], mult)
                    if ha == 0:
                        c_i = c_update(0)
                        nc.scalar.activation(tch[0][:], ct[0][:, 0:K, :], Tanh)
                # chain A tail first, then chain B's c/tanh/h behind it
                h_last = h_update(0)
                cb_i = c_update(1)
                bass_rust.add_dep_helper(cb_i.ins, h_last.ins, sync=False,
                                         reason="cB after hA")
                nc.scalar.activation(tch[1][:], ct[1][:, 0:K, :], Tanh)
                h_update(1)
                # window w+1 compute prep + window w+2 gather, spread across
                # this window's steps (all done by s=6 so the s=7 -> s=0
                # handoff is clean). X/bias matmuls are pinned behind this
                # step's recurrent burst so they fill the PE-idle tail.
                if w + 1 < NW:
                    if s == 0:
                        if w + 2 < NW:
                            gather(w + 2)
                        bias_mms(w + 1, 0, after=last_mm)
                    elif s == 1:
                        bias_mms(w + 1, 1, after=last_mm)
                    elif s <= 6:
                        n_x = M * K
                        lo = (s - 2) * n_x // 5
                        hi = (s - 1) * n_x // 5
                        x_mms(w + 1, 0, lo, hi, after=last_mm)
                        x_mms(w + 1, 1, lo, hi, after=last_mm)
            if w > 0:
                for ha in range(2):
                    ps.pop((w - 1, ha), None)
                embt.pop(w - 1, None)
    nc.finalize()
    return nc


def _prep_inputs(enc_inputs, h0, c0, embed, W_ih, W_hh, b_ih, b_hh, t_steps=T):
    """Host-side shard + layout prep. Returns list of per-core in_maps."""
    Wih_p = W_ih[_PERM].astype(np.float32).copy()
    Whh_p = W_hh[_PERM].astype(np.float32).copy()
    b_p = (b_ih + b_hh)[_PERM].astype(np.float32).copy()
    # g rows pre-scaled by 2: tanh(x) = 2*sigmoid(2x) - 1
    Wih_p[3 * H:] *= 2.0
    Whh_p[3 * H:] *= 2.0
    b_p[3 * H:] *= 2.0
    wih_t = np.ascontiguousarray(Wih_p.T).astype(np.float16)   # [H, 4H]
    whh_t = np.ascontiguousarray(Whh_p.T).astype(np.float16)
    # bias packed per PSUM bank (4 chunks per bank, 2 banks per half-window)
    bias = np.ascontiguousarray(
        b_p.astype(np.float16).reshape(2, 4, 128).transpose(1, 0, 2)
        .reshape(4, G4 // 4))
    table = embed.astype(np.float16)                           # [V, H]
    ind = np.zeros((4, 4 * S * HB), np.float16)
    for j in range(4):
        ind[j, S * HB * j:S * HB * (j + 1)] = 1.0

    NW = t_steps // S
    in_maps = []
    for c in range(N_CORES):
        wrapped = np.empty((128, NW, S * BL // 16), np.int16)
        for w in range(NW):
            # window's 256 indices: half A block then half B block, t-major
            blocks = []
            for ha in range(2):
                bs = slice(c * BL + HB * ha, c * BL + HB * (ha + 1))
                blocks.append(
                    enc_inputs[w * S:(w + 1) * S, bs].astype(np.int16).reshape(-1))
            flat = np.concatenate(blocks)                      # [256]
            w16 = flat.reshape(-1, 16).T                       # [16, 16]
            wrapped[:, w, :] = np.tile(w16, (8, 1))
        bs = slice(c * BL, (c + 1) * BL)
        h0t = np.empty((128, K, BL), np.float16)
        c0t = np.empty((128, K, BL), np.float32)
        for k in range(K):
            h0t[:, k, :] = h0[bs].T[128 * k:128 * (k + 1), :]
            c0t[:, k, :] = c0[bs].T[128 * k:128 * (k + 1), :]
        in_maps.append({
            "idx": np.ascontiguousarray(wrapped), "table": table,
            "wih_t": wih_t, "whh_t": whh_t,
            "bias": bias, "ind": ind, "h0t": h0t, "c0t": c0t,
        })
    return in_maps


def _unshard(results):
    h = np.empty((B, H), np.float32)
    c = np.empty((B, H), np.float32)
    for core, out in enumerate(results):
        bs = slice(core * BL, (core + 1) * BL)
        for k in range(K):
            h[bs, 128 * k:128 * (k + 1)] = out["h_out"][:, k, :].T
            c[bs, 128 * k:128 * (k + 1)] = out["c_out"][:, k, :].T
    return h, c


def kernel(enc_inputs, h0, c0, embed, W_ih, W_hh, b_ih, b_hh):
    from concourse.bass_utils import run_bass_kernel_spmd

    enc_inputs = np.asarray(enc_inputs)
    h0 = np.asarray(h0, dtype=np.float32)
    c0 = np.asarray(c0, dtype=np.float32)
    embed = np.asarray(embed, dtype=np.float32)
    W_ih = np.asarray(W_ih, dtype=np.float32)
    W_hh = np.asarray(W_hh, dtype=np.float32)
    b_ih = np.asarray(b_ih, dtype=np.float32)
    b_hh = np.asarray(b_hh, dtype=np.float32)

    nc = _build_bass()
    in_maps = _prep_inputs(enc_inputs, h0, c0, embed, W_ih, W_hh, b_ih, b_hh)
    res = run_bass_kernel_spmd(nc, in_maps, core_ids=list(range(N_CORES)))
    return _unshard(res.results)

